# revision 9
# baseline (speedup 1.0000x reference)
"""Multi-head attention (b=2, sq=skv=2048, dim=1024, 16 heads x 64) on 8 TRN2
NeuronCores.

Sharding: 2 heads per core (head-parallel across batch*heads), with the
matching tensor-parallel column slice of W_qkv and row slice of W_out.  Each
core computes a partial output projection over its 128 head-dims; the
all-reduce of the 8 partials (+ bias) happens on the host during unshard.

v2 design (ACT-bound steady state ~1us/step, PE trimmed to match):
  - scores: per (qt, j) one 2-bank PSUM tile [128 k, 2 h, 512 q]; the two
    heads' score matmuls (K=64) run concurrently in PE row-halves.
  - exp: one ACTIVATE per step over both heads (N=1024, scale 1/8 fused).
  - PV: col-tiled concurrent pair -- h0 -> acc[0:64], h1 -> acc[64:128]
    (tile_position (0,0)/(0,64) auto-derived), one PSUM bank per q-tile.
  - denominator: DVE running sum S += ex (bf16); at q-tile end two col-tiled
    M=1 ones-matmuls reduce S over k, DVE reciprocal -> bf16, two concurrent
    K=1 outer-product matmuls broadcast r to the 64 partitions, and the
    normalization is fused into the acc PSUM->SBUF copy (tensor_mul).
  - v is projected straight into natural [k-token, dim] layout by using the
    x chunk as the matmul stationary (no PE transposes, no identity).
  - emission: only chunk-0 projections precede attention(0); every other
    projection piece and outproj tile is drip-fed through per-step hooks
    (pre-hooks gate scores/PV inputs LOOKAHEAD steps early) so the ACT
    stream never starves and no long PE-only phase exists.
"""

import os
import sys

for _p in ("/opt/trn_rl_repo", "/root/.axon_site/_ro/trn_rl_repo"):
    if os.path.isdir(_p) and _p not in sys.path:
        sys.path.append(_p)

import ml_dtypes
import numpy as np

import concourse.bass as bass  # noqa: F401
import concourse.tile as tile
from concourse import bacc, mybir
from concourse.bass_utils import run_bass_kernel_spmd

B, SQ, SKV, DIM = 2, 2048, 2048, 1024
HEADS, DH = 16, 64
N_CORES = 8
HPC = HEADS // N_CORES  # heads per core = 2
HD = HPC * DH  # 128 head-dim rows per core
TOK = B * SQ  # 4096
KO = DIM // 128  # 8 contraction chunks of 128
SCALE = DH**-0.5

BF16 = mybir.dt.bfloat16
F32 = mybir.dt.float32

PCHUNK = 512  # token chunk in projections (contiguous per-chunk dram layout)
QTILE = 512  # q tile in attention
KTILE = 128  # k tile (scores psum partition dim)
NKT = SKV // KTILE  # 16
NQT = SQ // QTILE  # 4
NCPB = SQ // PCHUNK  # chunks per batch = 4

BF = ml_dtypes.bfloat16
Exp = mybir.ActivationFunctionType.Exp

LOOKAHEAD = 6


def build():
    nc = bacc.Bacc(
        "TRN2", target_bir_lowering=False, debug=False, num_devices=N_CORES
    )

    NCH = TOK // PCHUNK
    xqt_d = nc.dram_tensor("xqt", [NCH, 128, KO, PCHUNK], BF16, kind="ExternalInput")
    xkvt_d = nc.dram_tensor("xkvt", [NCH, 128, KO, PCHUNK], BF16, kind="ExternalInput")
    wq_d = nc.dram_tensor("wq", [128, KO, HD], BF16, kind="ExternalInput")
    wk_d = nc.dram_tensor("wk", [128, KO, HD], BF16, kind="ExternalInput")
    wv_d = nc.dram_tensor("wv", [128, KO, HD], BF16, kind="ExternalInput")
    wout_d = nc.dram_tensor("wout", [HD, DIM], BF16, kind="ExternalInput")
    out_d = nc.dram_tensor("out", [TOK, DIM], BF16, kind="ExternalOutput")

    xqt = xqt_d.ap()
    xkvt = xkvt_d.ap()

    with tile.TileContext(nc) as tc:
        with (
            tc.tile_pool(name="persist", bufs=1) as persist,
            tc.tile_pool(name="xin", bufs=6) as xin,
            tc.tile_pool(name="exps", bufs=8) as exps,
            tc.tile_pool(name="spool", bufs=2) as spool,
            tc.tile_pool(name="rbp", bufs=2) as rbp,
            tc.tile_pool(name="bcp", bufs=2) as bcp,
            tc.tile_pool(name="ost", bufs=3) as ost,
            tc.tile_pool(name="spsum", bufs=2, space="PSUM") as spsum,
            tc.tile_pool(name="accp", bufs=2, space="PSUM") as accp,
            tc.tile_pool(name="miscp", bufs=2, space="PSUM") as miscp,
        ):
            # --- weights / constants (gpsimd queue; ACT queue stays clean) ---
            wq_sb = persist.tile([128, KO, HD], BF16, tag="wq")
            nc.gpsimd.dma_start(wq_sb[:], wq_d.ap())
            wk_sb = persist.tile([128, KO, HD], BF16, tag="wk")
            nc.gpsimd.dma_start(wk_sb[:], wk_d.ap())
            wv_sb = persist.tile([128, KO, HD], BF16, tag="wv")
            nc.gpsimd.dma_start(wv_sb[:], wv_d.ap())
            wout_sb = persist.tile([HD, DIM], BF16, tag="wout")
            nc.gpsimd.dma_start(wout_sb[:], wout_d.ap())

            ones_col = persist.tile([128, 1], BF16, tag="ones_col")
            nc.vector.memset(ones_col[:], 1.0)
            ones_row = persist.tile([1, DH], BF16, tag="ones_row")
            nc.vector.memset(ones_row[:], 1.0)
            # prefetch the exp table set during the initial DMAs
            dummy = persist.tile([1, 8], F32, tag="dummy")
            nc.vector.memset(dummy[:], 0.0)
            nc.scalar.activation(dummy[:], dummy[:], Exp)

            qt_sb, kt_sb, vnat, outT = {}, {}, {}, {}
            for b in range(B):
                qt_sb[b] = persist.tile([HD, SQ], BF16, tag=f"qt{b}", name=f"qt{b}")
                kt_sb[b] = persist.tile([HD, SKV], BF16, tag=f"kt{b}", name=f"kt{b}")
                vnat[b] = persist.tile(
                    [128, NKT, HD], BF16, tag=f"vn{b}", name=f"vn{b}"
                )
                outT[b] = persist.tile([HD, SQ], BF16, tag=f"ot{b}", name=f"ot{b}")

            # ---------- projection pieces ----------
            kv_tiles = {0: {}, 1: {}}
            q_tiles = {0: {}, 1: {}}
            projps = {}

            def kv_load(b, tt):
                def go():
                    xt = xin.tile([128, KO, PCHUNK], BF16, tag="x")
                    nc.sync.dma_start(xt[:], xkvt[b * NCPB + tt])
                    kv_tiles[b][tt] = xt

                return go

            def q_load(b, tt):
                def go():
                    xt = xin.tile([128, KO, PCHUNK], BF16, tag="x")
                    nc.gpsimd.dma_start(xt[:], xqt[b * NCPB + tt])
                    q_tiles[b][tt] = xt

                return go

            def proj_half(dst_d, w_sb, src_d, b, tt, half):
                """4 of the 8 ko-accumulation matmuls; copy on second half."""

                def go():
                    if half == 0:
                        projps[0] = miscp.tile(
                            [128, PCHUNK], F32, tag="m", name="projp"
                        )
                    ps = projps[0]
                    xt = src_d[b][tt]
                    for ko in range(half * 4, half * 4 + 4):
                        nc.tensor.matmul(
                            ps[:],
                            w_sb[:, ko, :],
                            xt[:, ko, :],
                            start=(ko == 0),
                            stop=(ko == KO - 1),
                        )
                    if half == 1:
                        t0 = tt * PCHUNK
                        nc.vector.tensor_copy(
                            dst_d[b][:, t0 : t0 + PCHUNK], ps[:]
                        )

                return go

            def k_half(b, tt, half):
                return proj_half(kt_sb, wk_sb, kv_tiles, b, tt, half)

            def q_half(b, tt, half):
                return proj_half(qt_sb, wq_sb, q_tiles, b, tt, half)

            def v_piece(b, j):
                """Project v for k-tile j straight to [k-token, dim] layout."""

                def go():
                    xt = kv_tiles[b][j // 4]
                    ps = miscp.tile([128, PCHUNK], F32, tag="m", name="vnp")
                    t_sl = slice((j % 4) * KTILE, (j % 4 + 1) * KTILE)
                    for ko in range(KO):
                        nc.tensor.matmul(
                            ps[0:128, 0:KTILE],
                            xt[:, ko, t_sl],
                            wv_sb[:, ko, :],
                            start=(ko == 0),
                            stop=(ko == KO - 1),
                        )
                    nc.vector.tensor_copy(vnat[b][:, j, :], ps[0:128, 0:KTILE])

                return go

            # ---------- flush (denominator + normalization) ----------
            def flush(b, qt, acc, S):
                q_sl = slice(qt * QTILE, (qt + 1) * QTILE)
                dsum = miscp.tile([128, QTILE], F32, tag="m", name="dsum")
                for h in range(HPC):
                    nc.tensor.matmul(
                        dsum[h * 32 : h * 32 + 1, :],
                        ones_col[:],
                        S[:, h, :],
                        start=True,
                        stop=True,
                        skip_group_check=True,
                    )
                rb = rbp.tile([1, HPC, QTILE], BF16, tag="rb", name="rb")
                with nc.allow_low_precision(reason="bf16 softmax recip"):
                    for h in range(HPC):
                        nc.vector.reciprocal(
                            rb[0:1, h, :], dsum[h * 32 : h * 32 + 1, :]
                        )
                bc = miscp.tile([128, QTILE], F32, tag="m", name="bc")
                for h in range(HPC):
                    nc.tensor.matmul(
                        bc[h * DH : (h + 1) * DH, :],
                        ones_row[:],
                        rb[0:1, h, :],
                        start=True,
                        stop=True,
                        skip_group_check=True,
                    )
                # engines cannot read two PSUM operands; stage bc in SBUF
                bcs = bcp.tile([128, QTILE], BF16, tag="bc", name="bcs")
                nc.vector.tensor_copy(bcs[:], bc[:])
                for h in range(HPC):
                    h_sl = slice(h * DH, (h + 1) * DH)
                    nc.vector.tensor_mul(
                        outT[b][h_sl, q_sl], acc[h_sl, :], bcs[h_sl, :]
                    )

            # ---------- output projection ----------
            def outproj(b, tt, on_pool=True):
                def go():
                    t_sl = slice(tt * 128, (tt + 1) * 128)
                    ob = ost.tile([128, 2, 512], BF16, tag="o")
                    for nt in range(DIM // 512):
                        ps = miscp.tile([128, PCHUNK], F32, tag="m", name="projo")
                        nc.tensor.matmul(
                            ps[:],
                            outT[b][:, t_sl],
                            wout_sb[:, nt * 512 : (nt + 1) * 512],
                            start=True,
                            stop=True,
                        )
                        nc.vector.tensor_copy(ob[:, nt, :], ps[:])
                    nc.gpsimd.dma_start(
                        out_d.ap()[
                            b * SQ + tt * 128 : b * SQ + (tt + 1) * 128, :
                        ].rearrange("t (n c) -> t n c", n=2),
                        ob[:],
                    )

                return go

            # ---------- attention ----------
            def attention(b, pre_hooks, post_hooks):
                NT = NQT * NKT
                sps, st, accs = {}, {}, {}

                def emit_scores(t):
                    qt, j = divmod(t, NKT)
                    q_sl = slice(qt * QTILE, (qt + 1) * QTILE)
                    k_sl = slice(j * KTILE, (j + 1) * KTILE)
                    sp = spsum.tile([128, HPC, QTILE], F32, tag="s", name="sp")
                    sps[t] = sp
                    for h in range(HPC):
                        h_sl = slice(h * DH, (h + 1) * DH)
                        nc.tensor.matmul(
                            sp[:, h, :],
                            kt_sb[b][h_sl, k_sl],
                            qt_sb[b][h_sl, q_sl],
                            start=True,
                            stop=True,
                        )

                def emit_tail(t):
                    qt, j = divmod(t, NKT)
                    sp = sps.pop(t)
                    ex = exps.tile([128, HPC, QTILE], BF16, tag="e", name="ex")
                    nc.scalar.activation(ex[:], sp[:], Exp, scale=SCALE)
                    if j == 0:
                        accs[qt] = accp.tile(
                            [128, QTILE], F32, tag="acc", name="acc"
                        )
                        st[qt] = spool.tile(
                            [128, HPC, QTILE], BF16, tag="S", name="S"
                        )
                        nc.vector.tensor_copy(st[qt][:], ex[:])
                    else:
                        nc.vector.tensor_add(st[qt][:], st[qt][:], ex[:])
                    for h in range(HPC):
                        nc.tensor.matmul(
                            accs[qt][h * DH : (h + 1) * DH, :],
                            vnat[b][:, j, h * DH : (h + 1) * DH],
                            ex[:, h, :],
                            start=(j == 0),
                            stop=(j == NKT - 1),
                            skip_group_check=True,
                        )
                    if j == NKT - 1:
                        flush(b, qt, accs.pop(qt), st.pop(qt))
                    for fn in post_hooks.get((qt, j), []):
                        fn()

                for t in range(NT + LOOKAHEAD):
                    if t < NT:
                        for fn in pre_hooks.get(t, []):
                            fn()
                        emit_scores(t)
                    if t >= LOOKAHEAD:
                        emit_tail(t - LOOKAHEAD)

            # ---------- emission schedule ----------
            # lead-in: kv0+q0+kv1 loads, chunk-0 projections, vnat j0
            kv_load(0, 0)()
            q_load(0, 0)()
            kv_load(0, 1)()
            k_half(0, 0, 0)()
            k_half(0, 0, 1)()
            q_half(0, 0, 0)()
            q_half(0, 0, 1)()
            v_piece(0, 0)()
            v_piece(0, 1)()
            v_piece(0, 2)()
            v_piece(0, 3)()

            pre0 = {
                1: [k_half(0, 1, 0), k_half(0, 1, 1)],
                2: [v_piece(0, 4), v_piece(0, 5)],
                3: [v_piece(0, 6), v_piece(0, 7), kv_load(0, 2)],
                5: [k_half(0, 2, 0), k_half(0, 2, 1)],
                6: [v_piece(0, 8), v_piece(0, 9)],
                7: [v_piece(0, 10), v_piece(0, 11), kv_load(0, 3)],
                9: [k_half(0, 3, 0), k_half(0, 3, 1)],
                10: [v_piece(0, 12), v_piece(0, 13)],
                11: [v_piece(0, 14), v_piece(0, 15)],
                13: [q_load(0, 1)],
                15: [q_half(0, 1, 0), q_half(0, 1, 1)],
                17: [q_load(0, 2)],
                20: [q_half(0, 2, 0), q_half(0, 2, 1)],
                23: [q_load(0, 3)],
                26: [q_half(0, 3, 0), q_half(0, 3, 1)],
                30: [kv_load(1, 0)],
                33: [k_half(1, 0, 0), k_half(1, 0, 1)],
                35: [v_piece(1, 0), v_piece(1, 1)],
                36: [v_piece(1, 2), v_piece(1, 3), kv_load(1, 1)],
                39: [k_half(1, 1, 0), k_half(1, 1, 1)],
                41: [v_piece(1, 4), v_piece(1, 5)],
                42: [v_piece(1, 6), v_piece(1, 7), kv_load(1, 2)],
                45: [k_half(1, 2, 0), k_half(1, 2, 1)],
                47: [v_piece(1, 8), v_piece(1, 9)],
                48: [v_piece(1, 10), v_piece(1, 11), kv_load(1, 3)],
                51: [k_half(1, 3, 0), k_half(1, 3, 1)],
                53: [v_piece(1, 12), v_piece(1, 13)],
                54: [v_piece(1, 14), v_piece(1, 15)],
                55: [q_load(1, 0)],
                57: [q_half(1, 0, 0), q_half(1, 0, 1)],
                58: [q_load(1, 1)],
                59: [q_half(1, 1, 0), q_half(1, 1, 1)],
                60: [q_load(1, 2)],
                61: [q_half(1, 2, 0), q_half(1, 2, 1)],
                62: [q_load(1, 3)],
                63: [q_half(1, 3, 0), q_half(1, 3, 1)],
            }
            post0 = {}
            for qt in range(3):
                for i in range(4):
                    post0.setdefault((qt + 1, 2 + 3 * i), []).append(
                        outproj(0, qt * 4 + i)
                    )
            attention(0, pre0, post0)

            post1 = {}
            for i in range(4):
                post1.setdefault((0, 1 + 3 * i), []).append(outproj(0, 12 + i))
            for qt in range(3):
                for i in range(4):
                    post1.setdefault((qt + 1, 2 + 3 * i), []).append(
                        outproj(1, qt * 4 + i)
                    )
            attention(1, {}, post1)
            for i in range(4):
                outproj(1, 12 + i, on_pool=False)()

    nc.compile()
    return nc


def make_in_maps(x_q, x_kv, W_qkv, W_out):
    x_q = np.asarray(x_q, dtype=np.float32)
    x_kv = np.asarray(x_kv, dtype=np.float32)
    W_qkv = np.asarray(W_qkv, dtype=np.float32)
    W_out = np.asarray(W_out, dtype=np.float32)

    def chunk_tile(x):
        # [TOK, DIM] -> [n_chunks, 128, KO, PCHUNK] with D = ko*128 + p
        xt = x.reshape(TOK, DIM).T.reshape(KO, 128, TOK // PCHUNK, PCHUNK)
        return np.ascontiguousarray(xt.transpose(2, 1, 0, 3)).astype(BF)

    def w_tile(w):
        # [1024, HD] -> [128, KO, HD] with row = ko*128 + p
        return np.ascontiguousarray(
            w.reshape(KO, 128, HD).transpose(1, 0, 2)
        ).astype(BF)

    xqt = chunk_tile(x_q)
    xkvt = chunk_tile(x_kv)

    in_maps = []
    for c in range(N_CORES):
        cs = slice(c * HD, (c + 1) * HD)
        in_maps.append(
            {
                "xqt": xqt,
                "xkvt": xkvt,
                "wq": w_tile(W_qkv[:, cs]),
                "wk": w_tile(W_qkv[:, 1024:][:, cs]),
                "wv": w_tile(W_qkv[:, 2048:][:, cs]),
                "wout": np.ascontiguousarray(W_out[cs, :]).astype(BF),
            }
        )
    return in_maps


def combine(partials, b_out):
    """Sum the 8 per-core partial projections and add the bias."""
    acc = np.zeros((TOK, DIM), dtype=np.float32)
    for p in partials:
        acc += np.asarray(p, dtype=np.float32)
    acc += np.asarray(b_out, dtype=np.float32)
    return acc.reshape(B, SQ, DIM)


_STATE = {}


def _get_nc():
    if "nc" not in _STATE:
        _STATE["nc"] = build()
    return _STATE["nc"]


def run(x_q, x_kv, W_qkv, W_out, b_out, trace=False):
    nc = _get_nc()
    in_maps = make_in_maps(x_q, x_kv, W_qkv, W_out)
    res = run_bass_kernel_spmd(nc, in_maps, list(range(N_CORES)), trace=trace)
    out = combine([r["out"] for r in res.results], b_out)
    return out, res


def kernel(x_q, x_kv, W_qkv, W_out, b_out):
    out, _ = run(x_q, x_kv, W_qkv, W_out, b_out, trace=False)
    return out
